# revision 25
# baseline (speedup 1.0000x reference)
"""Trainium2 Bass kernel for BasicMotionEncoder (RAFT motion encoder).

Network (all stride-1 convs, NCHW, fp32 in/out):
    cor  = relu(conv1x1(corr, wc1, bc1))          # [B,256,H,W]
    cor  = relu(conv3x3(cor,  wc2, bc2, pad 1))   # [B,192,H,W]
    flo  = relu(conv7x7(flow, wf1, bf1, pad 3))   # [B,128,H,W]
    flo  = relu(conv3x3(flo,  wf2, bf2, pad 1))   # [B,64,H,W]
    out  = relu(conv3x3(cat(cor,flo), wo, bo, 1)) # [B,126,H,W]
    return cat(out, flow)                         # [B,128,H,W]

Sharding: pure data parallel, one image per NeuronCore (B=8, 8 cores).

Single-pass layout: every intermediate activation is held in SBUF in
bf16 for the WHOLE image (bf16 matmul operands run the PE at the same
1 row/cycle as float32r but halve SBUF, LDWEIGHTS and DMA cost; end-to-
end quantization error ~4e-3 against the fp32 reference).  With no row
passes there is no halo recompute: exactly 70 N=512 matmuls per 4-row
block-set, 1680 per image — the structural minimum for this dataflow.

Convs are PE matmuls with channels on the partition dim: for each tap
the shifted input window is a strided AP into a zero-padded SBUF image,
accumulated in PSUM fp32 over taps and k-tiles.  corr is pre-tiled on
the host into contiguous [128,4,128] DMA blocks; the 7x7 conv input is
a host-side im2col (partition = cin*49+dh*7+dw) so one K=98 matmul
computes a whole f1 block.  DMA queues: corr stream on sync, weights +
output on scalar, im2col stack on vector, f2's partition-shift copies
on gpsimd.  The final concat of `flow` into channels 126:128 happens on
the host.
"""

import numpy as np
import ml_dtypes

import concourse.mybir as mybir
import concourse.tile as tile
from concourse import bacc
from concourse.bass_utils import run_bass_kernel_spmd

H, W = 96, 128
CIN_CORR = 324
WP = W + 2  # pad-1 padded row width (3x3 convs)
NB = H // 4  # 24 four-row blocks
F32 = mybir.dt.float32
BF16 = mybir.dt.bfloat16
RELU = mybir.ActivationFunctionType.Relu
COPY = mybir.ActivationFunctionType.Copy

NR_C1 = H + 4  # flo1/cor1 buffer rows; row i = image row i-2
NR_CAT = H + 2  # catpad buffer rows;   row i = image row i-1
ZELEMS = 2 * NR_C1  # zeros tile length (col-border zeroing needs 2*rows)
PF = 4  # corr DMA issue runs this many blocks ahead of the c1 matmuls


def _zero_borders(nc, zsb, buf, zrows):
    """Zero the conv-padding bytes of a padded image buffer with small DVE
    copies from an SBUF zeros tile: cols 0 and 129 of every row (cols
    1..128 of computed rows are covered by the relu writes; the padding
    rows are zeroed whole), plus the vertical-padding zero rows.  On the
    idle Vector engine so the one-time setup doesn't serialize ahead of
    the relu ACTs on Scalar.  memset has no narrow encoding, strided DMA
    fills are pathologically slow in the DGEs, and bulk zero-fill DMAs
    steal enough SBUF write bandwidth to slow the PE's operand streaming
    — engine copies avoid all three."""
    nr = buf.shape[1]
    zv = zsb[:, 0:nr].rearrange("p (a b) -> p a b", b=1)
    for off in (0, WP - 1):
        nc.scalar.activation(buf[:, :, off : off + 1], zv, COPY)
    for zrow in zrows:
        nc.scalar.activation(
            buf[:, zrow : zrow + 1, :],
            zsb[:, 0:WP].rearrange("p (a b) -> p a b", a=1),
            COPY,
        )


def build_module():
    nc = bacc.Bacc(trn_type="TRN2", target_bir_lowering=False)
    corrp = nc.dram_tensor(
        "corrp", [NB, 3, 128, 4, 128], BF16, kind="ExternalInput"
    ).ap()
    stackh = nc.dram_tensor("stackh", [98, NR_C1, 128], BF16, kind="ExternalInput").ap()
    zeros = nc.dram_tensor("zeros", [128, ZELEMS], BF16, kind="ExternalInput").ap()
    wc1p = nc.dram_tensor("wc1p", [128, 3, 256], BF16, kind="ExternalInput").ap()
    wc2p = nc.dram_tensor("wc2p", [128, 9, 2, 192], BF16, kind="ExternalInput").ap()
    wf1p = nc.dram_tensor("wf1p", [98, 128], BF16, kind="ExternalInput").ap()
    wf2p = nc.dram_tensor("wf2p", [128, 9, 64], BF16, kind="ExternalInput").ap()
    wop = nc.dram_tensor("wop", [128, 2, 9, 126], BF16, kind="ExternalInput").ap()
    biasp = nc.dram_tensor("biasp", [128, 8], F32, kind="ExternalInput").ap()
    out = nc.dram_tensor("out", [126, H, W], F32, kind="ExternalOutput").ap()

    with tile.TileContext(nc) as tc:
        with (
            tc.tile_pool(name="wpool", bufs=1) as wpool,
            tc.tile_pool(name="pspool", space="PSUM", bufs=8) as pspool,
            tc.tile_pool(name="apool", bufs=1) as apool,
            tc.tile_pool(name="spool", bufs=3 * (PF + 1)) as spool,
            tc.tile_pool(name="fpool", bufs=3) as fpool,
            tc.tile_pool(name="opool", bufs=3) as opool,
        ):
            wc1s = wpool.tile([128, 3, 256], BF16, name="wc1s")
            wc2s = wpool.tile([128, 9, 2, 192], BF16, name="wc2s")
            wf1s = wpool.tile([98, 128], BF16, name="wf1s")
            wf2s = wpool.tile([128, 9, 64], BF16, name="wf2s")
            wos = wpool.tile([128, 2, 9, 126], BF16, name="wos")
            bs = wpool.tile([128, 8], F32, name="bs")
            zsb = wpool.tile([128, ZELEMS], BF16, name="zsb")
            scr = wpool.tile([128, 1], F32, name="scr")

            # whole-image activation buffers (bf16)
            stack2 = apool.tile([98, NR_C1, 128], BF16, name="stack2")
            flo1 = apool.tile([128, NR_C1, WP], BF16, name="flo1")
            cor1a = apool.tile([128, NR_C1, WP], BF16, name="cor1a")
            cor1b = apool.tile([128, NR_C1, WP], BF16, name="cor1b")
            catpad1 = apool.tile([128, NR_CAT, WP], BF16, name="catpad1")
            catpad2 = apool.tile([128, NR_CAT, WP], BF16, name="catpad2")

            # --- setup DMAs ---
            # corr prefetch for the first PF blocks leads everything on sync
            cts_q = {}

            def emit_c1_dma(bi):
                # kt2 (the 68-channel tail, smallest transfer) first: c1's
                # chain starts on it, so the first matmul of the block — and
                # at startup the whole kernel — waits on the least data
                cts = []
                for kt in (2, 0, 1):
                    kk = 128 if kt < 2 else CIN_CORR - 256
                    ct = spool.tile([128, 4, 128], BF16, tag="corr", name=f"ct_{bi}_{kt}")
                    nc.sync.dma_start(out=ct[0:kk], in_=corrp[bi, kt, 0:kk])
                    cts.append((kt, ct, kk))
                cts_q[bi] = cts

            # dma_start issue costs ~0.8us of the issuing ENGINE's queue, so
            # the scalar queue carries only the few weight issues (its FIFO
            # must reach the relu ACTs quickly) and the im2col chunks ride
            # the sync queue interleaved with the corr prefetch
            for j in range(PF):
                emit_c1_dma(j)
            for a, b in ((16, 32), (32, 64), (64, NR_C1)):
                nc.sync.dma_start(out=stack2[:, a:b, :], in_=stackh[:, a:b, :])
            nc.scalar.dma_start(out=zsb, in_=zeros)
            nc.scalar.dma_start(out=bs, in_=biasp)
            nc.scalar.dma_start(out=wc1s, in_=wc1p)
            nc.scalar.dma_start(out=wf1s, in_=wf1p)
            nc.scalar.dma_start(out=stack2[:, 0:8, :], in_=stackh[:, 0:8, :])
            nc.scalar.dma_start(out=stack2[:, 8:16, :], in_=stackh[:, 8:16, :])
            nc.scalar.dma_start(out=wf2s, in_=wf2p)
            # prewarm the Relu activation table off the critical path
            nc.scalar.activation(scr, bs[:, 7:8], RELU, bias=bs[:, 7:8])

            # only flo1's borders gate the first slots (f2 at idx 0); the
            # others are deferred into the loop just ahead of their consumers
            _zero_borders(nc, zsb, flo1, (1, NR_C1 - 2))
            nc.scalar.dma_start(out=wc2s, in_=wc2p)
            nc.scalar.dma_start(out=wos, in_=wop)

            # --- per-block stage emitters (rr = image-row block start) ---
            def emit_f1(rr):
                # 7x7 conv, 2 -> 128 channels, via host im2col (K=98)
                ps = pspool.tile([128, 4, 128], F32, tag="ps", name=f"psf1_{rr}")
                i = rr + 2
                nc.tensor.matmul(ps, wf1s, stack2[:, i : i + 4, :], start=True, stop=True)
                nc.scalar.activation(
                    flo1[:, i : i + 4, 1 : 1 + W], ps, RELU, bias=bs[:, 4:5]
                )

            def emit_f2(cc):
                # 3x3 conv, 128 -> 64 channels -> catpad2[64:128].  Column
                # tile_position crashes the exec unit, so accumulate at psum
                # partitions 0:64 and partition-shift into catpad2[64:128]
                # with an SBUF->SBUF DMA.
                ps = pspool.tile([128, 4, 128], F32, tag="ps", name=f"psf2_{cc}")
                k = 0
                for dh in range(3):
                    for dw in range(3):
                        i = cc + 1 + dh
                        nc.tensor.matmul(
                            ps[0:64],
                            wf2s[:, k, :],
                            flo1[:, i : i + 4, dw : dw + 128],
                            start=(k == 0),
                            stop=(k == 8),
                        )
                        k += 1
                flo2t = fpool.tile([64, 4, 128], BF16, tag="flo2t", name=f"flo2t_{cc}")
                nc.scalar.activation(flo2t, ps[0:64], RELU, bias=bs[0:64, 5:6])
                nc.sync.dma_start(
                    out=catpad2[64:128, cc + 1 : cc + 5, 1 : 1 + W], in_=flo2t
                )

            def emit_c1_mm(rr):
                # consecutive matmuls into the SAME psum bank chain at
                # ~218ns; switching banks costs a ~110ns pipeline drain, so
                # run each psum chain contiguously instead of alternating
                cts = cts_q.pop(rr // 4)
                ps0 = pspool.tile([128, 4, 128], F32, tag="ps", name=f"psc1a_{rr}")
                ps1 = pspool.tile([128, 4, 128], F32, tag="ps", name=f"psc1b_{rr}")
                for j, (kt, ct, kk) in enumerate(cts):
                    nc.tensor.matmul(
                        ps0, wc1s[0:kk, kt, 0:128], ct[0:kk], start=(j == 0), stop=(j == 2)
                    )
                for j, (kt, ct, kk) in enumerate(cts):
                    nc.tensor.matmul(
                        ps1, wc1s[0:kk, kt, 128:256], ct[0:kk], start=(j == 0), stop=(j == 2)
                    )
                r = rr + 2
                nc.scalar.activation(cor1a[:, r : r + 4, 1 : 1 + W], ps0, RELU, bias=bs[:, 0:1])
                nc.scalar.activation(cor1b[:, r : r + 4, 1 : 1 + W], ps1, RELU, bias=bs[:, 1:2])

            def emit_c2(cc):
                ps0 = pspool.tile([128, 4, 128], F32, tag="ps", name=f"psc2a_{cc}")
                ps1 = pspool.tile([128, 4, 128], F32, tag="ps", name=f"psc2b_{cc}")
                for mt, (ps, mm) in enumerate(((ps0, slice(0, 128)), (ps1[0:64], slice(128, 192)))):
                    k = 0
                    for kt, src_ in enumerate((cor1a, cor1b)):
                        for dh in range(3):
                            for dw in range(3):
                                i = cc + 1 + dh
                                nc.tensor.matmul(
                                    ps,
                                    wc2s[:, 3 * dh + dw, kt, mm],
                                    src_[:, i : i + 4, dw : dw + 128],
                                    start=(k == 0),
                                    stop=(k == 17),
                                )
                                k += 1
                r = cc + 1
                nc.scalar.activation(catpad1[:, r : r + 4, 1 : 1 + W], ps0, RELU, bias=bs[:, 2:3])
                nc.scalar.activation(
                    catpad2[0:64, r : r + 4, 1 : 1 + W], ps1[0:64], RELU, bias=bs[0:64, 3:4]
                )

            def emit_o(oo):
                ps = pspool.tile([128, 4, 128], F32, tag="ps", name=f"pso_{oo}")
                k = 0
                for kt, src_ in enumerate((catpad1, catpad2)):
                    for dh in range(3):
                        for dw in range(3):
                            i = oo + dh
                            nc.tensor.matmul(
                                ps[0:126],
                                wos[:, kt, 3 * dh + dw, :],
                                src_[:, i : i + 4, dw : dw + 128],
                                start=(k == 0),
                                stop=(k == 17),
                            )
                            k += 1
                ob = opool.tile([128, 4, 128], F32, tag="ob", name=f"ob_{oo}")
                nc.scalar.activation(ob[0:126], ps[0:126], RELU, bias=bs[0:126, 6:7])
                nc.scalar.dma_start(out=out[:, oo : oo + 4, :], in_=ob[0:126])

            # --- software-pipelined interleave ---
            # c1 has only 6 matmuls per block against ~1us of corr DMA; run
            # alone it starves the PE.  Interleaving f2 (9), c2 (36) and o
            # (18) behind it keeps the PE dense while corr streams.
            # c1 first in each slot: the PE is in-order, and at startup the
            # corr tile for c1(0) lands (sync DGE) well before f1's im2col
            # chunk (scalar DGE), so c1 leading lets the PE start ~8us sooner
            for idx in range(NB + 4):
                if idx + PF < NB:
                    emit_c1_dma(idx + PF)
                if idx < NB:
                    emit_c1_mm(4 * idx)
                    emit_f1(4 * idx)
                if 0 <= idx - 1 < NB:
                    emit_f2(4 * (idx - 1))
                if 0 <= idx - 2 < NB:
                    emit_c2(4 * (idx - 2))
                if 0 <= idx - 4 < NB:
                    emit_o(4 * (idx - 4))
                if idx == 0:
                    _zero_borders(nc, zsb, cor1a, (1, NR_C1 - 2))
                    _zero_borders(nc, zsb, cor1b, (1, NR_C1 - 2))
                elif idx == 1:
                    _zero_borders(nc, zsb, catpad1, (0, NR_CAT - 1))
                    _zero_borders(nc, zsb, catpad2, (0, NR_CAT - 1))
    nc.compile()
    return nc


def pack_params(wc1, bc1, wc2, bc2, wf1, bf1, wf2, bf2, wo, bo):
    """Host-side repack of OIHW conv weights into the lhsT layouts the
    kernel's matmuls read ([K partitions, ..., M]), in bf16."""
    f = np.float32
    wc1p = np.zeros((128, 3, 256), f)
    w = wc1[:, :, 0, 0]  # [256, 324]
    for kt in range(3):
        kk = min(128, CIN_CORR - kt * 128)
        wc1p[0:kk, kt, :] = w[:, kt * 128 : kt * 128 + kk].T
    wc2p = np.zeros((128, 9, 2, 192), f)
    for dh in range(3):
        for dw in range(3):
            for kt in range(2):
                wc2p[:, 3 * dh + dw, kt, :] = wc2[:, kt * 128 : kt * 128 + 128, dh, dw].T
    wf1p = np.zeros((98, 128), f)
    for cin in range(2):
        for dh in range(7):
            for dw in range(7):
                wf1p[cin * 49 + dh * 7 + dw, :] = wf1[:, cin, dh, dw]
    wf2p = np.zeros((128, 9, 64), f)
    for dh in range(3):
        for dw in range(3):
            wf2p[:, 3 * dh + dw, :] = wf2[:, :, dh, dw].T
    wop = np.zeros((128, 2, 9, 126), f)
    for dh in range(3):
        for dw in range(3):
            tap = 3 * dh + dw
            wop[:, 0, tap, :] = wo[:, 0:128, dh, dw].T
            wop[0:64, 1, tap, :] = wo[:, 128:192, dh, dw].T
            wop[64:128, 1, tap, :] = wo[:, 192:256, dh, dw].T
    biasp = np.zeros((128, 8), f)
    biasp[:, 0] = bc1[0:128]
    biasp[:, 1] = bc1[128:256]
    biasp[:, 2] = bc2[0:128]
    biasp[0:64, 3] = bc2[128:192]
    biasp[:, 4] = bf1
    biasp[0:64, 5] = bf2
    biasp[0:126, 6] = bo
    bf = ml_dtypes.bfloat16
    return {
        "wc1p": wc1p.astype(bf),
        "wc2p": wc2p.astype(bf),
        "wf1p": wf1p.astype(bf),
        "wf2p": wf2p.astype(bf),
        "wop": wop.astype(bf),
        "biasp": biasp,
    }


def build_stackh(flow_b):
    """Full f1 im2col: [98, H+4, 128], partition cin*49+dh*7+dw holds the
    zero-padded (pad 3) flow image shifted by (dh, dw); row i <-> f1 output
    row i-2."""
    fz = np.zeros((2, H + 10, W + 6), np.float32)
    fz[:, 5 : 5 + H, 3 : 3 + W] = flow_b
    s = np.empty((98, H + 4, 128), np.float32)
    for cin in range(2):
        for dh in range(7):
            for dw in range(7):
                s[cin * 49 + dh * 7 + dw] = fz[cin, dh : dh + H + 4, dw : dw + 128]
    return s.astype(ml_dtypes.bfloat16)


def pack_corr(corr_b):
    """Pre-tile one image's corr into contiguous DMA blocks:
    [NB, 3, 128, 4, 128] bf16, block bi / k-tile kt holding channels
    kt*128.. of image rows 4bi..4bi+4 (unused tail partitions zero)."""
    c = np.zeros((384, H, W), np.float32)
    c[0:CIN_CORR] = corr_b
    c = c.reshape(3, 128, NB, 4, W).transpose(2, 0, 1, 3, 4)
    return np.ascontiguousarray(c).astype(ml_dtypes.bfloat16)


_MODULE = None


def _get_module():
    global _MODULE
    if _MODULE is None:
        _MODULE = build_module()
    return _MODULE


def make_in_maps(**inputs):
    a = {
        k: np.ascontiguousarray(np.asarray(v), dtype=np.float32)
        for k, v in inputs.items()
    }
    packed = pack_params(
        a["wc1"], a["bc1"], a["wc2"], a["bc2"], a["wf1"], a["bf1"],
        a["wf2"], a["bf2"], a["wo"], a["bo"],
    )
    zeros = np.zeros((128, ZELEMS), ml_dtypes.bfloat16)
    in_maps = []
    for b in range(8):
        m = dict(packed)
        m["corrp"] = pack_corr(a["corr"][b])
        m["stackh"] = build_stackh(a["flow"][b])
        m["zeros"] = zeros
        in_maps.append(m)
    return in_maps, a["flow"]


def assemble_output(results, flow):
    out = np.empty((8, 128, H, W), np.float32)
    for b in range(8):
        out[b, :126] = results[b]["out"]
        out[b, 126:] = flow[b]
    return out


def run(trace=False, **inputs):
    in_maps, flow = make_in_maps(**inputs)
    nc = _get_module()
    res = run_bass_kernel_spmd(nc, in_maps, core_ids=list(range(8)), trace=trace)
    return assemble_output(res.results, flow), res


def kernel(**inputs):
    out, _ = run(trace=False, **inputs)
    return out


# revision 27
# speedup vs baseline: 1.0044x; 1.0044x over previous
"""Trainium2 Bass kernel for BasicMotionEncoder (RAFT motion encoder).

Network (all stride-1 convs, NCHW, fp32 in/out):
    cor  = relu(conv1x1(corr, wc1, bc1))          # [B,256,H,W]
    cor  = relu(conv3x3(cor,  wc2, bc2, pad 1))   # [B,192,H,W]
    flo  = relu(conv7x7(flow, wf1, bf1, pad 3))   # [B,128,H,W]
    flo  = relu(conv3x3(flo,  wf2, bf2, pad 1))   # [B,64,H,W]
    out  = relu(conv3x3(cat(cor,flo), wo, bo, 1)) # [B,126,H,W]
    return cat(out, flow)                         # [B,128,H,W]

Sharding: pure data parallel, one image per NeuronCore (B=8, 8 cores).

Single-pass layout: every intermediate activation is held in SBUF in
bf16 for the WHOLE image (bf16 matmul operands run the PE at the same
1 row/cycle as float32r but halve SBUF, LDWEIGHTS and DMA cost; end-to-
end quantization error ~4e-3 against the fp32 reference).  With no row
passes there is no halo recompute: exactly 70 N=512 matmuls per 4-row
block-set, 1680 per image — the structural minimum for this dataflow.

Convs are PE matmuls with channels on the partition dim: for each tap
the shifted input window is a strided AP into a zero-padded SBUF image,
accumulated in PSUM fp32 over taps and k-tiles.  Matmuls are emitted so
consecutive instructions accumulate into the SAME psum bank (a bank
switch costs a ~110ns PE pipeline drain; chained bf16 matmuls run at
~218ns for N=512).  corr is pre-tiled on the host into contiguous
[128,4,128] DMA blocks; the 7x7 conv input is a host-side im2col
(partition = cin*49+dh*7+dw) so one K=98 matmul computes a whole f1
block.  DMA queues: corr stream + bulk im2col on sync, weights + early
im2col + output on scalar (each dma_start issue costs ~0.8us of the
issuing engine's FIFO, so scalar carries few issues ahead of its relu
ACTs).  The final concat of `flow` into channels 126:128 happens on the
host.
"""

import numpy as np
import ml_dtypes

import concourse.mybir as mybir
import concourse.tile as tile
from concourse import bacc
from concourse.bass_utils import run_bass_kernel_spmd

H, W = 96, 128
CIN_CORR = 324
WP = W + 2  # pad-1 padded row width (3x3 convs)
NB = H // 4  # 24 four-row blocks
F32 = mybir.dt.float32
BF16 = mybir.dt.bfloat16
RELU = mybir.ActivationFunctionType.Relu
COPY = mybir.ActivationFunctionType.Copy

NR_C1 = H + 4  # flo1/cor1 buffer rows; row i = image row i-2
NR_CAT = H + 2  # catpad buffer rows;   row i = image row i-1
ZELEMS = 2 * NR_C1  # zeros tile length (col-border zeroing needs 2*rows)
PF = 4  # corr DMA issue runs this many blocks ahead of the c1 matmuls


def _zero_borders(nc, zsb, buf, zrows):
    """Zero the conv-padding bytes of a padded image buffer with small ACT
    copies from an SBUF zeros tile: cols 0 and 129 of every row (cols
    1..128 of computed rows are covered by the relu writes; the padding
    rows are zeroed whole), plus the vertical-padding zero rows.  Must
    stay on the Scalar engine: moving these writes to Vector makes every
    matmul that reads the buffer carry a cross-engine dependency and run
    ~20% slower.  memset has no narrow encoding, strided DMA fills are
    pathologically slow in the DGEs, and bulk zero-fill DMAs steal
    enough SBUF write bandwidth to slow the PE's operand streaming — ACT
    copies avoid all three."""
    nr = buf.shape[1]
    zv = zsb[:, 0:nr].rearrange("p (a b) -> p a b", b=1)
    for off in (0, WP - 1):
        nc.scalar.activation(buf[:, :, off : off + 1], zv, COPY)
    for zrow in zrows:
        nc.scalar.activation(
            buf[:, zrow : zrow + 1, :],
            zsb[:, 0:WP].rearrange("p (a b) -> p a b", a=1),
            COPY,
        )


def build_module():
    nc = bacc.Bacc(trn_type="TRN2", target_bir_lowering=False)
    corrp = nc.dram_tensor(
        "corrp", [NB, 3, 128, 4, 128], BF16, kind="ExternalInput"
    ).ap()
    stackh = nc.dram_tensor("stackh", [98, NR_C1, 128], BF16, kind="ExternalInput").ap()
    zeros = nc.dram_tensor("zeros", [128, ZELEMS], BF16, kind="ExternalInput").ap()
    wc1p = nc.dram_tensor("wc1p", [128, 3, 256], BF16, kind="ExternalInput").ap()
    wc2p = nc.dram_tensor("wc2p", [128, 9, 2, 192], BF16, kind="ExternalInput").ap()
    wf1p = nc.dram_tensor("wf1p", [98, 128], BF16, kind="ExternalInput").ap()
    wf2p = nc.dram_tensor("wf2p", [128, 9, 64], BF16, kind="ExternalInput").ap()
    wop = nc.dram_tensor("wop", [128, 2, 9, 126], BF16, kind="ExternalInput").ap()
    biasp = nc.dram_tensor("biasp", [128, 8], F32, kind="ExternalInput").ap()
    out = nc.dram_tensor("out", [126, H, W], F32, kind="ExternalOutput").ap()

    with tile.TileContext(nc) as tc:
        with (
            tc.tile_pool(name="wpool", bufs=1) as wpool,
            tc.tile_pool(name="pspool", space="PSUM", bufs=8) as pspool,
            tc.tile_pool(name="apool", bufs=1) as apool,
            tc.tile_pool(name="spool", bufs=3 * (PF + 1)) as spool,
            tc.tile_pool(name="fpool", bufs=3) as fpool,
            tc.tile_pool(name="opool", bufs=3) as opool,
        ):
            wc1s = wpool.tile([128, 3, 256], BF16, name="wc1s")
            wc2s = wpool.tile([128, 9, 2, 192], BF16, name="wc2s")
            wf1s = wpool.tile([98, 128], BF16, name="wf1s")
            wf2s = wpool.tile([128, 9, 64], BF16, name="wf2s")
            wos = wpool.tile([128, 2, 9, 126], BF16, name="wos")
            bs = wpool.tile([128, 8], F32, name="bs")
            zsb = wpool.tile([128, ZELEMS], BF16, name="zsb")
            scr = wpool.tile([128, 1], F32, name="scr")

            # whole-image activation buffers (bf16)
            stack2 = apool.tile([98, NR_C1, 128], BF16, name="stack2")
            flo1 = apool.tile([128, NR_C1, WP], BF16, name="flo1")
            cor1a = apool.tile([128, NR_C1, WP], BF16, name="cor1a")
            cor1b = apool.tile([128, NR_C1, WP], BF16, name="cor1b")
            catpad1 = apool.tile([128, NR_CAT, WP], BF16, name="catpad1")
            catpad2 = apool.tile([128, NR_CAT, WP], BF16, name="catpad2")

            # --- setup DMAs ---
            # corr prefetch for the first PF blocks leads everything on sync
            cts_q = {}

            def emit_c1_dma(bi):
                # kt2 (the 68-channel tail, smallest transfer) first: c1's
                # chain starts on it, so the first matmul of the block — and
                # at startup the whole kernel — waits on the least data
                cts = []
                for kt in (2, 0, 1):
                    kk = 128 if kt < 2 else CIN_CORR - 256
                    ct = spool.tile([128, 4, 128], BF16, tag="corr", name=f"ct_{bi}_{kt}")
                    nc.sync.dma_start(out=ct[0:kk], in_=corrp[bi, kt, 0:kk])
                    cts.append((kt, ct, kk))
                cts_q[bi] = cts

            # dma_start issue costs ~0.8us of the issuing ENGINE's queue, so
            # the scalar queue carries only the few weight issues (its FIFO
            # must reach the relu ACTs quickly) and the im2col chunks ride
            # the sync queue interleaved with the corr prefetch
            for j in range(PF):
                emit_c1_dma(j)
            for a, b in ((16, 32), (32, 64), (64, NR_C1)):
                nc.sync.dma_start(out=stack2[:, a:b, :], in_=stackh[:, a:b, :])
            nc.scalar.dma_start(out=zsb, in_=zeros)
            nc.scalar.dma_start(out=bs, in_=biasp)
            nc.scalar.dma_start(out=wc1s, in_=wc1p)
            nc.scalar.dma_start(out=wf1s, in_=wf1p)
            nc.scalar.dma_start(out=stack2[:, 0:8, :], in_=stackh[:, 0:8, :])
            nc.scalar.dma_start(out=stack2[:, 8:16, :], in_=stackh[:, 8:16, :])
            nc.scalar.dma_start(out=wf2s, in_=wf2p)
            # prewarm the Relu activation table off the critical path
            nc.scalar.activation(scr, bs[:, 7:8], RELU, bias=bs[:, 7:8])

            # only flo1's borders gate the first slots (f2 at idx 0); the
            # others are deferred into the loop just ahead of their consumers
            _zero_borders(nc, zsb, flo1, (1, NR_C1 - 2))
            nc.scalar.dma_start(out=wc2s, in_=wc2p)
            nc.scalar.dma_start(out=wos, in_=wop)

            # --- per-block stage emitters (rr = image-row block start) ---
            def emit_f1(rr):
                # 7x7 conv, 2 -> 128 channels, via host im2col (K=98)
                ps = pspool.tile([128, 4, 128], F32, tag="ps", name=f"psf1_{rr}")
                i = rr + 2
                nc.tensor.matmul(ps, wf1s, stack2[:, i : i + 4, :], start=True, stop=True)
                nc.scalar.activation(
                    flo1[:, i : i + 4, 1 : 1 + W], ps, RELU, bias=bs[:, 4:5]
                )

            def emit_f2(cc):
                # 3x3 conv, 128 -> 64 channels -> catpad2[64:128].  Column
                # tile_position crashes the exec unit, so accumulate at psum
                # partitions 0:64 and partition-shift into catpad2[64:128]
                # with an SBUF->SBUF DMA.
                ps = pspool.tile([128, 4, 128], F32, tag="ps", name=f"psf2_{cc}")
                k = 0
                for dh in range(3):
                    for dw in range(3):
                        i = cc + 1 + dh
                        nc.tensor.matmul(
                            ps[0:64],
                            wf2s[:, k, :],
                            flo1[:, i : i + 4, dw : dw + 128],
                            start=(k == 0),
                            stop=(k == 8),
                        )
                        k += 1
                flo2t = fpool.tile([64, 4, 128], BF16, tag="flo2t", name=f"flo2t_{cc}")
                nc.scalar.activation(flo2t, ps[0:64], RELU, bias=bs[0:64, 5:6])
                nc.sync.dma_start(
                    out=catpad2[64:128, cc + 1 : cc + 5, 1 : 1 + W], in_=flo2t
                )

            def emit_c1_mm(rr):
                # consecutive matmuls into the SAME psum bank chain at
                # ~218ns; switching banks costs a ~110ns pipeline drain, so
                # run each psum chain contiguously instead of alternating
                cts = cts_q.pop(rr // 4)
                ps0 = pspool.tile([128, 4, 128], F32, tag="ps", name=f"psc1a_{rr}")
                ps1 = pspool.tile([128, 4, 128], F32, tag="ps", name=f"psc1b_{rr}")
                for j, (kt, ct, kk) in enumerate(cts):
                    nc.tensor.matmul(
                        ps0, wc1s[0:kk, kt, 0:128], ct[0:kk], start=(j == 0), stop=(j == 2)
                    )
                for j, (kt, ct, kk) in enumerate(cts):
                    nc.tensor.matmul(
                        ps1, wc1s[0:kk, kt, 128:256], ct[0:kk], start=(j == 0), stop=(j == 2)
                    )
                r = rr + 2
                nc.scalar.activation(cor1a[:, r : r + 4, 1 : 1 + W], ps0, RELU, bias=bs[:, 0:1])
                nc.scalar.activation(cor1b[:, r : r + 4, 1 : 1 + W], ps1, RELU, bias=bs[:, 1:2])

            def emit_c2(cc):
                ps0 = pspool.tile([128, 4, 128], F32, tag="ps", name=f"psc2a_{cc}")
                ps1 = pspool.tile([128, 4, 128], F32, tag="ps", name=f"psc2b_{cc}")
                for mt, (ps, mm) in enumerate(((ps0, slice(0, 128)), (ps1[0:64], slice(128, 192)))):
                    k = 0
                    for kt, src_ in enumerate((cor1a, cor1b)):
                        for dh in range(3):
                            for dw in range(3):
                                i = cc + 1 + dh
                                nc.tensor.matmul(
                                    ps,
                                    wc2s[:, 3 * dh + dw, kt, mm],
                                    src_[:, i : i + 4, dw : dw + 128],
                                    start=(k == 0),
                                    stop=(k == 17),
                                )
                                k += 1
                r = cc + 1
                nc.scalar.activation(catpad1[:, r : r + 4, 1 : 1 + W], ps0, RELU, bias=bs[:, 2:3])
                nc.scalar.activation(
                    catpad2[0:64, r : r + 4, 1 : 1 + W], ps1[0:64], RELU, bias=bs[0:64, 3:4]
                )

            def emit_o(oo):
                ps = pspool.tile([128, 4, 128], F32, tag="ps", name=f"pso_{oo}")
                k = 0
                for kt, src_ in enumerate((catpad1, catpad2)):
                    for dh in range(3):
                        for dw in range(3):
                            i = oo + dh
                            nc.tensor.matmul(
                                ps[0:126],
                                wos[:, kt, 3 * dh + dw, :],
                                src_[:, i : i + 4, dw : dw + 128],
                                start=(k == 0),
                                stop=(k == 17),
                            )
                            k += 1
                ob = opool.tile([128, 4, 128], F32, tag="ob", name=f"ob_{oo}")
                nc.scalar.activation(ob[0:126], ps[0:126], RELU, bias=bs[0:126, 6:7])
                nc.scalar.dma_start(out=out[:, oo : oo + 4, :], in_=ob[0:126])

            # --- software-pipelined interleave ---
            # c1 has only 6 matmuls per block against ~1us of corr DMA; run
            # alone it starves the PE.  Interleaving f2 (9), c2 (36) and o
            # (18) behind it keeps the PE dense while corr streams.
            # c1 first in each slot: the PE is in-order, and at startup the
            # corr tile for c1(0) lands (sync DGE) well before f1's im2col
            # chunk (scalar DGE), so c1 leading lets the PE start ~8us sooner
            for idx in range(NB + 4):
                if idx + PF < NB:
                    emit_c1_dma(idx + PF)
                if idx < NB:
                    emit_c1_mm(4 * idx)
                    emit_f1(4 * idx)
                if 0 <= idx - 1 < NB:
                    emit_f2(4 * (idx - 1))
                if 0 <= idx - 2 < NB:
                    emit_c2(4 * (idx - 2))
                if 0 <= idx - 4 < NB:
                    emit_o(4 * (idx - 4))
                if idx == 0:
                    _zero_borders(nc, zsb, cor1a, (1, NR_C1 - 2))
                    _zero_borders(nc, zsb, cor1b, (1, NR_C1 - 2))
                elif idx == 1:
                    _zero_borders(nc, zsb, catpad1, (0, NR_CAT - 1))
                    _zero_borders(nc, zsb, catpad2, (0, NR_CAT - 1))
    nc.compile()
    return nc


def pack_params(wc1, bc1, wc2, bc2, wf1, bf1, wf2, bf2, wo, bo):
    """Host-side repack of OIHW conv weights into the lhsT layouts the
    kernel's matmuls read ([K partitions, ..., M]), in bf16."""
    f = np.float32
    wc1p = np.zeros((128, 3, 256), f)
    w = wc1[:, :, 0, 0]  # [256, 324]
    for kt in range(3):
        kk = min(128, CIN_CORR - kt * 128)
        wc1p[0:kk, kt, :] = w[:, kt * 128 : kt * 128 + kk].T
    wc2p = np.zeros((128, 9, 2, 192), f)
    for dh in range(3):
        for dw in range(3):
            for kt in range(2):
                wc2p[:, 3 * dh + dw, kt, :] = wc2[:, kt * 128 : kt * 128 + 128, dh, dw].T
    wf1p = np.zeros((98, 128), f)
    for cin in range(2):
        for dh in range(7):
            for dw in range(7):
                wf1p[cin * 49 + dh * 7 + dw, :] = wf1[:, cin, dh, dw]
    wf2p = np.zeros((128, 9, 64), f)
    for dh in range(3):
        for dw in range(3):
            wf2p[:, 3 * dh + dw, :] = wf2[:, :, dh, dw].T
    wop = np.zeros((128, 2, 9, 126), f)
    for dh in range(3):
        for dw in range(3):
            tap = 3 * dh + dw
            wop[:, 0, tap, :] = wo[:, 0:128, dh, dw].T
            wop[0:64, 1, tap, :] = wo[:, 128:192, dh, dw].T
            wop[64:128, 1, tap, :] = wo[:, 192:256, dh, dw].T
    biasp = np.zeros((128, 8), f)
    biasp[:, 0] = bc1[0:128]
    biasp[:, 1] = bc1[128:256]
    biasp[:, 2] = bc2[0:128]
    biasp[0:64, 3] = bc2[128:192]
    biasp[:, 4] = bf1
    biasp[0:64, 5] = bf2
    biasp[0:126, 6] = bo
    bf = ml_dtypes.bfloat16
    return {
        "wc1p": wc1p.astype(bf),
        "wc2p": wc2p.astype(bf),
        "wf1p": wf1p.astype(bf),
        "wf2p": wf2p.astype(bf),
        "wop": wop.astype(bf),
        "biasp": biasp,
    }


def build_stackh(flow_b):
    """Full f1 im2col: [98, H+4, 128], partition cin*49+dh*7+dw holds the
    zero-padded (pad 3) flow image shifted by (dh, dw); row i <-> f1 output
    row i-2."""
    fz = np.zeros((2, H + 10, W + 6), np.float32)
    fz[:, 5 : 5 + H, 3 : 3 + W] = flow_b
    s = np.empty((98, H + 4, 128), np.float32)
    for cin in range(2):
        for dh in range(7):
            for dw in range(7):
                s[cin * 49 + dh * 7 + dw] = fz[cin, dh : dh + H + 4, dw : dw + 128]
    return s.astype(ml_dtypes.bfloat16)


def pack_corr(corr_b):
    """Pre-tile one image's corr into contiguous DMA blocks:
    [NB, 3, 128, 4, 128] bf16, block bi / k-tile kt holding channels
    kt*128.. of image rows 4bi..4bi+4 (unused tail partitions zero)."""
    c = np.zeros((384, H, W), np.float32)
    c[0:CIN_CORR] = corr_b
    c = c.reshape(3, 128, NB, 4, W).transpose(2, 0, 1, 3, 4)
    return np.ascontiguousarray(c).astype(ml_dtypes.bfloat16)


_MODULE = None


def _get_module():
    global _MODULE
    if _MODULE is None:
        _MODULE = build_module()
    return _MODULE


def make_in_maps(**inputs):
    a = {
        k: np.ascontiguousarray(np.asarray(v), dtype=np.float32)
        for k, v in inputs.items()
    }
    packed = pack_params(
        a["wc1"], a["bc1"], a["wc2"], a["bc2"], a["wf1"], a["bf1"],
        a["wf2"], a["bf2"], a["wo"], a["bo"],
    )
    zeros = np.zeros((128, ZELEMS), ml_dtypes.bfloat16)
    in_maps = []
    for b in range(8):
        m = dict(packed)
        m["corrp"] = pack_corr(a["corr"][b])
        m["stackh"] = build_stackh(a["flow"][b])
        m["zeros"] = zeros
        in_maps.append(m)
    return in_maps, a["flow"]


def assemble_output(results, flow):
    out = np.empty((8, 128, H, W), np.float32)
    for b in range(8):
        out[b, :126] = results[b]["out"]
        out[b, 126:] = flow[b]
    return out


def run(trace=False, **inputs):
    in_maps, flow = make_in_maps(**inputs)
    nc = _get_module()
    res = run_bass_kernel_spmd(nc, in_maps, core_ids=list(range(8)), trace=trace)
    return assemble_output(res.results, flow), res


def kernel(**inputs):
    out, _ = run(trace=False, **inputs)
    return out
